# revision 1
# baseline (speedup 1.0000x reference)
"""ROIAlign (torchvision semantics, aligned=True) on 8 Trainium2 cores.

Problem (hardcoded): input [4, 256, 256, 256] f32 (NCHW), rois [512, 5]
(batch_idx, x1, y1, x2, y2), output [512, 256, 7, 7] f32.
output_size=7x7, sampling_ratio=2, spatial_scale=0.25, aligned=True.

Strategy
--------
Host: permute the feature map to NHWC so each pixel's 256 channels are a
contiguous 1KB run; view each image as 32768 "pixel pairs" of 512 f32 (2KB),
so any pair is addressable with an int16 index (0..32767).  For every RoI the
4*4 bilinear corner taps of the 14x14 sample grid are reduced to a deduped,
sorted list of pixel-pair ids plus two sparse weight matrices (one per pixel
half) that fold bilinear weights, validity masking and 2x2 average pooling.

RoIs are routed by image to a pair of cores (4 images x 2 cores), sorted by
gather size so every core's slot s has a similar size; the per-slot shapes
(J_s = ceil(maxcount/128)) are the max over cores, so a single SPMD kernel
serves all 8 cores.

Device (per slot): dma_gather pulls count_s pairs (runtime register; tail
padded with -1) into SBUF [128, J_s, 512]; 2*J_s accumulating fp32r matmuls
(lhsT = weight [128,49], rhs = gathered [128,256]) produce PSUM [49, 256]
= the RoI's [7,7,256] output; ACT copies to SBUF and it is DMA'd out.

Host reassembles [slot, 49, 256] -> [512, 256, 7, 7].
"""
import os
import numpy as np

N, C, H, W = 4, 256, 256, 256
B = 512
PH = PW = 7
SR = 2
SCALE = 0.25
NS = PH * SR          # 14 samples per axis
NTAP = NS * 2         # 28 taps per axis
NBIN = PH * PW        # 49
NPAIR = H * (W // 2)  # 32768 pairs per image
ELEM = 2 * C          # 512 f32 per pair

LAST_RESULTS = None   # BassKernelResults of the most recent run (for test.py)
_NC_CACHE = {}


def _interp_1d(c, size):
    """Exact replica of reference._interp_1d in float32 numpy."""
    valid = (c >= -1.0) & (c <= size)
    c = np.maximum(c, np.float32(0.0))
    lo_f = np.floor(c)
    lo = np.minimum(lo_f.astype(np.int32), size - 1)
    hi = np.minimum(lo + 1, size - 1)
    c_adj = np.where(lo_f >= size - 1, np.float32(size - 1), c)
    frac = (c_adj - lo.astype(np.float32)).astype(np.float32)
    return lo, hi, frac, valid


def _roi_meta(rois):
    """Per-roi tap rows/cols + weights, replicating the reference math."""
    r = rois.astype(np.float32)
    bidx = r[:, 0].astype(np.int32)
    off = np.float32(0.5)
    x1 = r[:, 1] * np.float32(SCALE) - off
    y1 = r[:, 2] * np.float32(SCALE) - off
    x2 = r[:, 3] * np.float32(SCALE) - off
    y2 = r[:, 4] * np.float32(SCALE) - off
    roi_w, roi_h = x2 - x1, y2 - y1
    bin_w, bin_h = roi_w / np.float32(PW), roi_h / np.float32(PH)

    offs = ((np.arange(SR, dtype=np.float32) + np.float32(0.5)) / np.float32(SR))
    grid = (np.arange(PH, dtype=np.float32)[:, None] + offs[None, :])  # [7, 2]
    ys = (y1[:, None, None] + grid[None] * bin_h[:, None, None]).reshape(B, NS)
    xs = (x1[:, None, None] + grid[None] * bin_w[:, None, None]).reshape(B, NS)

    yl, yh, fy, vy = _interp_1d(ys, H)
    xl, xh, fx, vx = _interp_1d(xs, W)

    # taps along each axis: (lo, 1-frac), (hi, frac); validity and the /2 of
    # the per-axis pooling mean folded in.
    vyf = vy.astype(np.float32) * np.float32(0.5)
    vxf = vx.astype(np.float32) * np.float32(0.5)
    Yt = np.stack([yl, yh], -1).reshape(B, NTAP)                 # [B, 28] rows
    Xt = np.stack([xl, xh], -1).reshape(B, NTAP)                 # [B, 28] cols
    Wy = (np.stack([1.0 - fy, fy], -1) * vyf[..., None]).reshape(B, NTAP)
    Wx = (np.stack([1.0 - fx, fx], -1) * vxf[..., None]).reshape(B, NTAP)
    return bidx, Yt, Xt, Wy, Wx


# bin id for tap index t (t = sample*2 + corner): ph = t // 4
_BIN_Y = (np.arange(NTAP) // 4)
_BIN_OF_TAP = (_BIN_Y[:, None] * PW + (np.arange(NTAP) // 4)[None, :]).ravel()


def _roi_tables(Yt, Xt, Wy, Wx):
    """For one roi: (pair_ids[int16 sorted], W0, W1 [cnt, 49] float32)."""
    pair = (Yt.astype(np.int32) * (W // 2))[:, None] + (Xt // 2)[None, :]
    half = (Xt & 1)
    w = Wy[:, None].astype(np.float64) * Wx[None, :].astype(np.float64)
    pair = pair.ravel()
    half = np.broadcast_to(half[None, :], (NTAP, NTAP)).ravel()
    w = w.ravel()
    uniq, inv = np.unique(pair, return_inverse=True)
    cnt = len(uniq)
    W01 = np.zeros((2, cnt, NBIN), dtype=np.float64)
    np.add.at(W01, (half, inv, _BIN_OF_TAP), w)
    return uniq.astype(np.int16), W01[0].astype(np.float32), W01[1].astype(np.float32)


def _build_nc(J_prof):
    """SPMD kernel for a per-slot chunk-count profile (tuple of ints)."""
    import concourse.bacc as bacc
    import concourse.tile as tile
    import concourse.mybir as mybir

    K = len(J_prof)
    total_mm = int(2 * sum(J_prof))
    total_cols = int(8 * sum(J_prof))
    f32 = mybir.dt.float32
    f32r = mybir.dt.float32r

    nc = bacc.Bacc("TRN2", debug=False)
    img_d = nc.declare_dram_parameter("img", [NPAIR, ELEM], f32r, isOutput=False)
    idx_d = nc.declare_dram_parameter("idx", [128, total_cols], mybir.dt.int16, isOutput=False)
    wts_d = nc.declare_dram_parameter("wts", [128, total_mm, NBIN], f32r, isOutput=False)
    cnt_d = nc.declare_dram_parameter("cnt", [1, K], mybir.dt.int32, isOutput=False)
    out_d = nc.declare_dram_parameter("out", [K, NBIN, C], f32, isOutput=True)

    with tile.TileContext(nc) as tc:
        with tc.tile_pool(name="meta", bufs=1) as meta_pool, \
             tc.tile_pool(name="g", bufs=3) as gpool, \
             tc.tile_pool(name="w", bufs=3) as wpool, \
             tc.tile_pool(name="o", bufs=4) as opool, \
             tc.tile_pool(name="ps", bufs=6, space="PSUM") as pspool:
            idx_t = meta_pool.tile([128, total_cols], mybir.dt.int16)
            cnt_t = meta_pool.tile([1, K], mybir.dt.int32)
            nc.sync.dma_start(out=idx_t[:], in_=idx_d[:])
            nc.sync.dma_start(out=cnt_t[:], in_=cnt_d[:])

            icol = imm = 0
            for s in range(K):
                J = J_prof[s]
                g_t = gpool.tile([128, J, ELEM], f32r, tag="g")
                w_t = wpool.tile([128, 2 * J, NBIN], f32r, tag="w")
                nc.sync.dma_start(out=w_t[:], in_=wts_d[:, imm:imm + 2 * J, :])
                reg = nc.gpsimd.alloc_register()
                nc.gpsimd.reg_load(reg, cnt_t[:, s:s + 1])
                nc.gpsimd.dma_gather(
                    g_t[:], img_d[:], idx_t[:, icol:icol + 8 * J],
                    J * 128, reg, ELEM,
                )
                ps = pspool.tile([NBIN, C], f32, space="PSUM", tag="ps")
                for j in range(J):
                    nc.tensor.matmul(
                        ps[:], lhsT=w_t[:, 2 * j, :], rhs=g_t[:, j, 0:C],
                        start=(j == 0), stop=False,
                    )
                    nc.tensor.matmul(
                        ps[:], lhsT=w_t[:, 2 * j + 1, :], rhs=g_t[:, j, C:ELEM],
                        start=False, stop=(j == J - 1),
                    )
                o_t = opool.tile([NBIN, C], f32, tag="o")
                nc.scalar.copy(o_t[:], ps[:])
                nc.scalar.dma_start(out=out_d[s], in_=o_t[:])
                icol += 8 * J
                imm += 2 * J
    nc.compile()
    return nc


def kernel(input, rois):
    global LAST_RESULTS
    from concourse.bass_utils import run_bass_kernel_spmd

    input = np.ascontiguousarray(input, dtype=np.float32)
    rois = np.asarray(rois, dtype=np.float32)

    # NHWC, viewed as [N, 32768 pairs, 512]
    img_nhwc = np.ascontiguousarray(input.transpose(0, 2, 3, 1))
    img_pairs = img_nhwc.reshape(N, NPAIR, ELEM)

    bidx, Yt, Xt, Wy, Wx = _roi_meta(rois)

    tables = [_roi_tables(Yt[r], Xt[r], Wy[r], Wx[r]) for r in range(B)]
    cnts = np.array([len(t[0]) for t in tables])

    # ---- route rois: image n -> cores 2n, 2n+1, sorted by gather size ----
    core_rois = [[] for _ in range(8)]
    for n in range(N):
        rs = np.where(bidx == n)[0]
        rs = rs[np.argsort(-cnts[rs], kind="stable")]
        core_rois[2 * n] = list(rs[0::2])
        core_rois[2 * n + 1] = list(rs[1::2])

    K = max(len(cr) for cr in core_rois)
    # per-slot J profile = max over cores (pad slots count as 1 pair)
    J_prof = []
    for s in range(K):
        m = max((cnts[cr[s]] if s < len(cr) else 1) for cr in core_rois)
        J_prof.append(int(-(-int(m) // 128)))
    J_prof = tuple(J_prof)
    total_mm = 2 * sum(J_prof)
    total_cols = 8 * sum(J_prof)

    # ---- build per-core input arrays ----
    in_maps = []
    for c in range(8):
        cr = core_rois[c]
        idx_arr = np.full((16, total_cols), -1, dtype=np.int16)
        cnt_arr = np.zeros((1, K), dtype=np.int32)
        wts_arr = np.zeros((128, total_mm, NBIN), dtype=np.float32)
        icol = imm = 0
        for s in range(K):
            J = J_prof[s]
            if s < len(cr):
                ids, W0, W1 = tables[cr[s]]
                m = len(ids)
            else:
                ids = np.zeros(1, dtype=np.int16)
                W0 = W1 = np.zeros((1, NBIN), dtype=np.float32)
                m = 1
            cnt_arr[0, s] = m
            pad = np.full(J * 128, -1, dtype=np.int16)
            pad[:m] = ids
            idx_arr[:, icol:icol + 8 * J] = pad.reshape(-1, 16).T
            w0 = np.zeros((J * 128, NBIN), dtype=np.float32)
            w1 = np.zeros((J * 128, NBIN), dtype=np.float32)
            w0[:m] = W0
            w1[:m] = W1
            for j in range(J):
                wts_arr[:, imm + 2 * j, :] = w0[j * 128:(j + 1) * 128]
                wts_arr[:, imm + 2 * j + 1, :] = w1[j * 128:(j + 1) * 128]
            icol += 8 * J
            imm += 2 * J
        in_maps.append({
            "img": img_pairs[c // 2],
            "idx": np.tile(idx_arr, (8, 1)),
            "wts": wts_arr,
            "cnt": cnt_arr,
        })

    if J_prof not in _NC_CACHE:
        _NC_CACHE[J_prof] = _build_nc(J_prof)
    nc = _NC_CACHE[J_prof]

    res = run_bass_kernel_spmd(nc, in_maps, core_ids=list(range(8)))
    LAST_RESULTS = res

    out = np.zeros((B, C, PH, PW), dtype=np.float32)
    for c in range(8):
        ores = res.results[c]["out"]  # [K, 49, 256]
        for s, rid in enumerate(core_rois[c]):
            out[rid] = ores[s].reshape(PH, PW, C).transpose(2, 0, 1)
    return out


# revision 2
# speedup vs baseline: 1.1087x; 1.1087x over previous
"""ROIAlign (torchvision semantics, aligned=True) on 8 Trainium2 cores.

Problem (hardcoded): input [4, 256, 256, 256] f32 (NCHW), rois [512, 5]
(batch_idx, x1, y1, x2, y2) -> output [512, 256, 7, 7] f32.
output_size=7x7, sampling_ratio=2, spatial_scale=0.25, aligned=True.

Strategy
--------
Host: permute the feature map to NHWC so channels are contiguous per pixel.
Each image is addressed at "pixel pair" granularity (32768 pairs of 512
elements) so one int16 index reaches any location.  Per RoI, the 28x28
bilinear corner taps are covered by row-segments of EP pixels (starting on
an even pixel), giving a short list of segment indices plus a dense weight
tensor [segments, EP, 49] that folds bilinear weights, validity and average
pooling (out[bin, c] = sum_seg sum_px W[seg, px, bin] * img[seg, px, c]).

RoIs are routed by image to a pair of cores, sorted by segment count so a
single SPMD program (per-slot shapes = max over cores) serves all 8 cores.

Device per slot: dma_gather pulls count_s segments (runtime register; tail
-1) into SBUF [128, J, EP*256]; J*EP accumulating matmuls (lhsT = weights
[128, 49], rhs = gathered [128, 256], contraction over the 128 segments of
a chunk) produce PSUM [49, 256] = that RoI's [7, 7, 256] output; ACT copies
to SBUF, DMA writes out [slot, 49, 256].  Host reassembles to NCHW bins.
"""
import numpy as np

N, C, H, W = 4, 256, 256, 256
B = 512
PH = PW = 7
SR = 2
SCALE = 0.25
NS = PH * SR          # 14 samples per axis
NTAP = NS * 2         # 28 taps per axis
NBIN = PH * PW        # 49
NPAIR = H * (W // 2)  # 32768 pairs per image
PAIR_ELEMS = 2 * C    # 512 elements per pair

EP = 6                # pixels per gathered segment (even)
USE_BF16 = False      # False -> float32r everywhere

LAST_RESULTS = None   # BassKernelResults of the most recent run (for test.py)
_NC_CACHE = {}


def _interp_1d(c, size):
    """Exact replica of reference._interp_1d in float32 numpy."""
    valid = (c >= -1.0) & (c <= size)
    c = np.maximum(c, np.float32(0.0))
    lo_f = np.floor(c)
    lo = np.minimum(lo_f.astype(np.int32), size - 1)
    hi = np.minimum(lo + 1, size - 1)
    c_adj = np.where(lo_f >= size - 1, np.float32(size - 1), c)
    frac = (c_adj - lo.astype(np.float32)).astype(np.float32)
    return lo, hi, frac, valid


def _roi_meta(rois):
    """Per-roi tap rows/cols + weights, replicating the reference math."""
    r = rois.astype(np.float32)
    bidx = r[:, 0].astype(np.int32)
    off = np.float32(0.5)
    x1 = r[:, 1] * np.float32(SCALE) - off
    y1 = r[:, 2] * np.float32(SCALE) - off
    x2 = r[:, 3] * np.float32(SCALE) - off
    y2 = r[:, 4] * np.float32(SCALE) - off
    roi_w, roi_h = x2 - x1, y2 - y1
    bin_w, bin_h = roi_w / np.float32(PW), roi_h / np.float32(PH)

    offs = ((np.arange(SR, dtype=np.float32) + np.float32(0.5)) / np.float32(SR))
    grid = (np.arange(PH, dtype=np.float32)[:, None] + offs[None, :])  # [7, 2]
    ys = (y1[:, None, None] + grid[None] * bin_h[:, None, None]).reshape(B, NS)
    xs = (x1[:, None, None] + grid[None] * bin_w[:, None, None]).reshape(B, NS)

    yl, yh, fy, vy = _interp_1d(ys, H)
    xl, xh, fx, vx = _interp_1d(xs, W)

    vyf = vy.astype(np.float32) * np.float32(0.5)   # fold the /2 pooling mean
    vxf = vx.astype(np.float32) * np.float32(0.5)
    Yt = np.stack([yl, yh], -1).reshape(B, NTAP)                 # [B, 28] rows
    Xt = np.stack([xl, xh], -1).reshape(B, NTAP)                 # [B, 28] cols
    Wy = (np.stack([1.0 - fy, fy], -1) * vyf[..., None]).reshape(B, NTAP)
    Wx = (np.stack([1.0 - fx, fx], -1) * vxf[..., None]).reshape(B, NTAP)
    return bidx, Yt, Xt, Wy, Wx


# bin id for tap-pair (ty, tx): ph = ty // 4, pw = tx // 4
_BIN_OF_TAP = ((np.arange(NTAP) // 4)[:, None] * PW + (np.arange(NTAP) // 4)[None, :]).ravel()


def _roi_tables(Yt, Xt, Wy, Wx, ep):
    """One roi -> (segment pair-ids int16 sorted-by-construction,
    W [nseg, ep, 49] float32)."""
    rows = np.unique(Yt)                        # distinct image rows, sorted
    xs = np.unique(Xt)                          # distinct tap columns, sorted
    # greedy cover of xs by [start, start+ep) windows, start even, <= W-ep
    starts = []
    cov = -1
    for x in xs:
        if x > cov:
            s = min(int(x) & ~1, W - ep)
            starts.append(s)
            cov = s + ep - 1
    starts = np.asarray(starts, dtype=np.int32)
    ncov = len(starts)
    # map each tap column -> cover window (first window covering it)
    cov_id = np.searchsorted(starts, Xt, side="right") - 1
    # (windows are disjoint except possibly near the right edge; the
    # found window always covers the tap since cov >= tap for greedy)
    slice_in = Xt - starts[cov_id]
    assert (slice_in >= 0).all() and (slice_in < ep).all()

    row_pos = np.searchsorted(rows, Yt)
    nseg = len(rows) * ncov
    # segment i = row_pos * ncov + cov_id  (row-major; ids ascending)
    seg_ids = (rows[:, None] * (W // 2) + (starts[None, :] >> 1)).ravel()
    tap_seg = (row_pos[:, None] * ncov + cov_id[None, :]).ravel()
    tap_slice = np.broadcast_to(slice_in[None, :], (NTAP, NTAP)).ravel()
    w = (Wy[:, None].astype(np.float64) * Wx[None, :].astype(np.float64)).ravel()

    Wt = np.zeros((nseg, ep, NBIN), dtype=np.float64)
    np.add.at(Wt, (tap_seg, tap_slice, _BIN_OF_TAP), w)
    return seg_ids.astype(np.int16), Wt.astype(np.float32)


def _build_nc(J_prof, ep, use_bf16):
    """SPMD kernel for a per-slot chunk-count profile (tuple of ints)."""
    import concourse.bacc as bacc
    import concourse.tile as tile
    import concourse.mybir as mybir

    K = len(J_prof)
    total_mm = int(ep * sum(J_prof))
    total_cols = int(8 * sum(J_prof))
    ELEM = ep * C
    dt = mybir.dt.bfloat16 if use_bf16 else mybir.dt.float32r
    f32 = mybir.dt.float32
    NROW = NPAIR - ep // 2 + 1

    nc = bacc.Bacc("TRN2", debug=False)
    img_d = nc.declare_dram_parameter("img", [NPAIR, PAIR_ELEMS], dt, isOutput=False)
    idx_d = nc.declare_dram_parameter("idx", [128, total_cols], mybir.dt.int16, isOutput=False)
    wts_d = nc.declare_dram_parameter("wts", [128, total_mm, NBIN], dt, isOutput=False)
    cnt_d = nc.declare_dram_parameter("cnt", [1, K], mybir.dt.int32, isOutput=False)
    out_d = nc.declare_dram_parameter("out", [K, NBIN, C], f32, isOutput=True)

    def img_view():
        ap = img_d[:]
        ap.ap.clear()
        ap.ap.extend([[PAIR_ELEMS, NROW], [1, ELEM]])
        return ap

    with tile.TileContext(nc) as tc:
        with tc.tile_pool(name="meta", bufs=1) as meta_pool, \
             tc.tile_pool(name="g", bufs=3) as gpool, \
             tc.tile_pool(name="w", bufs=3) as wpool, \
             tc.tile_pool(name="o", bufs=4) as opool, \
             tc.tile_pool(name="ps", bufs=6, space="PSUM") as pspool:
            idx_t = meta_pool.tile([128, total_cols], mybir.dt.int16)
            cnt_t = meta_pool.tile([1, K], mybir.dt.int32)
            nc.sync.dma_start(out=idx_t[:], in_=idx_d[:])
            nc.sync.dma_start(out=cnt_t[:], in_=cnt_d[:])

            icol = imm = 0
            for s in range(K):
                J = J_prof[s]
                g_t = gpool.tile([128, J, ELEM], dt, tag="g")
                w_t = wpool.tile([128, ep * J, NBIN], dt, tag="w")
                nc.sync.dma_start(out=w_t[:], in_=wts_d[:, imm:imm + ep * J, :])
                reg = nc.gpsimd.alloc_register()
                nc.gpsimd.reg_load(reg, cnt_t[:, s:s + 1])
                nc.gpsimd.dma_gather(
                    g_t[:], img_view(), idx_t[:, icol:icol + 8 * J],
                    J * 128, reg, ELEM, elem_step=PAIR_ELEMS,
                )
                ps = pspool.tile([NBIN, C], f32, space="PSUM", tag="ps")
                nmm = J * ep
                m = 0
                for j in range(J):
                    for px in range(ep):
                        nc.tensor.matmul(
                            ps[:], lhsT=w_t[:, j * ep + px, :],
                            rhs=g_t[:, j, px * C:(px + 1) * C],
                            start=(m == 0), stop=(m == nmm - 1),
                        )
                        m += 1
                o_t = opool.tile([NBIN, C], f32, tag="o")
                nc.scalar.copy(o_t[:], ps[:])
                nc.scalar.dma_start(out=out_d[s], in_=o_t[:])
                icol += 8 * J
                imm += ep * J
    nc.compile()
    return nc


def kernel(input, rois):
    global LAST_RESULTS
    from concourse.bass_utils import run_bass_kernel_spmd
    import ml_dtypes

    input = np.ascontiguousarray(input, dtype=np.float32)
    rois = np.asarray(rois, dtype=np.float32)

    img_nhwc = np.ascontiguousarray(input.transpose(0, 2, 3, 1))
    np_dt = ml_dtypes.bfloat16 if USE_BF16 else np.float32
    img_pairs = img_nhwc.reshape(N, NPAIR, PAIR_ELEMS).astype(np_dt)

    bidx, Yt, Xt, Wy, Wx = _roi_meta(rois)
    tables = [_roi_tables(Yt[r], Xt[r], Wy[r], Wx[r], EP) for r in range(B)]
    cnts = np.array([len(t[0]) for t in tables])

    # ---- route rois: image n -> cores 2n, 2n+1, sorted by gather size ----
    core_rois = [[] for _ in range(8)]
    for n in range(N):
        rs = np.where(bidx == n)[0]
        rs = rs[np.argsort(-cnts[rs], kind="stable")]
        core_rois[2 * n] = list(rs[0::2])
        core_rois[2 * n + 1] = list(rs[1::2])

    K = max(len(cr) for cr in core_rois)
    J_prof = []
    for s in range(K):
        m = max((cnts[cr[s]] if s < len(cr) else 1) for cr in core_rois)
        J_prof.append(int(-(-int(m) // 128)))
    J_prof = tuple(J_prof)
    total_mm = EP * sum(J_prof)
    total_cols = 8 * sum(J_prof)

    # ---- per-core input arrays ----
    in_maps = []
    for c in range(8):
        cr = core_rois[c]
        idx_arr = np.full((16, total_cols), -1, dtype=np.int16)
        cnt_arr = np.zeros((1, K), dtype=np.int32)
        wts_arr = np.zeros((128, total_mm, NBIN), dtype=np_dt)
        icol = imm = 0
        for s in range(K):
            J = J_prof[s]
            if s < len(cr):
                ids, Wt = tables[cr[s]]
                m = len(ids)
            else:
                ids = np.zeros(1, dtype=np.int16)
                Wt = np.zeros((1, EP, NBIN), dtype=np.float32)
                m = 1
            cnt_arr[0, s] = m
            pad = np.full(J * 128, -1, dtype=np.int16)
            pad[:m] = ids
            idx_arr[:, icol:icol + 8 * J] = pad.reshape(-1, 16).T
            wt = np.zeros((J * 128, EP, NBIN), dtype=np.float32)
            wt[:m] = Wt
            # -> [128, J*EP, 49] slot block: block (j, px) = wt[j*128:(j+1)*128, px]
            blk = wt.reshape(J, 128, EP, NBIN).transpose(1, 0, 2, 3).reshape(128, J * EP, NBIN)
            wts_arr[:, imm:imm + EP * J, :] = blk.astype(np_dt)
            icol += 8 * J
            imm += EP * J
        in_maps.append({
            "img": img_pairs[c // 2],
            "idx": np.tile(idx_arr, (8, 1)),
            "wts": wts_arr,
            "cnt": cnt_arr,
        })

    key = (J_prof, EP, USE_BF16)
    if key not in _NC_CACHE:
        _NC_CACHE[key] = _build_nc(J_prof, EP, USE_BF16)
    nc = _NC_CACHE[key]

    res = run_bass_kernel_spmd(nc, in_maps, core_ids=list(range(8)))
    LAST_RESULTS = res

    out = np.zeros((B, C, PH, PW), dtype=np.float32)
    for c in range(8):
        ores = res.results[c]["out"]  # [K, 49, 256]
        for s, rid in enumerate(core_rois[c]):
            out[rid] = ores[s].reshape(PH, PW, C).transpose(2, 0, 1)
    return out


# revision 4
# speedup vs baseline: 1.2707x; 1.1461x over previous
"""ROIAlign (torchvision semantics, aligned=True) on 8 Trainium2 cores.

Problem (hardcoded): input [4, 256, 256, 256] f32 (NCHW), rois [512, 5]
(batch_idx, x1, y1, x2, y2) -> output [512, 256, 7, 7] f32.
output_size=7x7, sampling_ratio=2, spatial_scale=0.25, aligned=True.

Strategy
--------
Host: permute the feature map to NHWC so channels are contiguous per pixel.
Each image is addressed at "pixel pair" granularity (32768 pairs of 512
elements) so one int16 index reaches any location.  Per RoI, the 28x28
bilinear corner taps are covered by row-segments of EP pixels (starting on
an even pixel), giving a short list of segment indices plus a dense weight
tensor [segments, EP, 49] that folds bilinear weights, validity and average
pooling (out[bin, c] = sum_seg sum_px W[seg, px, bin] * img[seg, px, c]).

RoIs are routed by image to a pair of cores, sorted by segment count so a
single SPMD program (per-slot shapes = max over cores) serves all 8 cores.

Device per slot: dma_gather pulls count_s segments (runtime register; tail
-1) into SBUF [128, J, EP*256]; J*EP accumulating matmuls (lhsT = weights
[128, 49], rhs = gathered [128, 256], contraction over the 128 segments of
a chunk) produce PSUM [49, 256] = that RoI's [7, 7, 256] output; ACT copies
to SBUF, DMA writes out [slot, 49, 256].  Host reassembles to NCHW bins.
"""
import numpy as np

N, C, H, W = 4, 256, 256, 256
B = 512
PH = PW = 7
SR = 2
SCALE = 0.25
NS = PH * SR          # 14 samples per axis
NTAP = NS * 2         # 28 taps per axis
NBIN = PH * PW        # 49
NPAIR = H * (W // 2)  # 32768 pairs per image
PAIR_ELEMS = 2 * C    # 512 elements per pair

import os
EP = int(os.environ.get("ROIALIGN_EP", "8"))     # pixels per gathered segment (even)
USE_BF16 = os.environ.get("ROIALIGN_BF16", "1") == "1"
WBATCH = 4            # slots per weight-DMA / output-DMA batch

LAST_RESULTS = None   # BassKernelResults of the most recent run (for test.py)
_NC_CACHE = {}


def _interp_1d(c, size):
    """Exact replica of reference._interp_1d in float32 numpy."""
    valid = (c >= -1.0) & (c <= size)
    c = np.maximum(c, np.float32(0.0))
    lo_f = np.floor(c)
    lo = np.minimum(lo_f.astype(np.int32), size - 1)
    hi = np.minimum(lo + 1, size - 1)
    c_adj = np.where(lo_f >= size - 1, np.float32(size - 1), c)
    frac = (c_adj - lo.astype(np.float32)).astype(np.float32)
    return lo, hi, frac, valid


def _roi_meta(rois):
    """Per-roi tap rows/cols + weights, replicating the reference math."""
    r = rois.astype(np.float32)
    bidx = r[:, 0].astype(np.int32)
    off = np.float32(0.5)
    x1 = r[:, 1] * np.float32(SCALE) - off
    y1 = r[:, 2] * np.float32(SCALE) - off
    x2 = r[:, 3] * np.float32(SCALE) - off
    y2 = r[:, 4] * np.float32(SCALE) - off
    roi_w, roi_h = x2 - x1, y2 - y1
    bin_w, bin_h = roi_w / np.float32(PW), roi_h / np.float32(PH)

    offs = ((np.arange(SR, dtype=np.float32) + np.float32(0.5)) / np.float32(SR))
    grid = (np.arange(PH, dtype=np.float32)[:, None] + offs[None, :])  # [7, 2]
    ys = (y1[:, None, None] + grid[None] * bin_h[:, None, None]).reshape(B, NS)
    xs = (x1[:, None, None] + grid[None] * bin_w[:, None, None]).reshape(B, NS)

    yl, yh, fy, vy = _interp_1d(ys, H)
    xl, xh, fx, vx = _interp_1d(xs, W)

    vyf = vy.astype(np.float32) * np.float32(0.5)   # fold the /2 pooling mean
    vxf = vx.astype(np.float32) * np.float32(0.5)
    Yt = np.stack([yl, yh], -1).reshape(B, NTAP)                 # [B, 28] rows
    Xt = np.stack([xl, xh], -1).reshape(B, NTAP)                 # [B, 28] cols
    Wy = (np.stack([1.0 - fy, fy], -1) * vyf[..., None]).reshape(B, NTAP)
    Wx = (np.stack([1.0 - fx, fx], -1) * vxf[..., None]).reshape(B, NTAP)
    return bidx, Yt, Xt, Wy, Wx


# bin id for tap-pair (ty, tx): ph = ty // 4, pw = tx // 4
_BIN_OF_TAP = ((np.arange(NTAP) // 4)[:, None] * PW + (np.arange(NTAP) // 4)[None, :]).ravel()


def _roi_tables(Yt, Xt, Wy, Wx, ep):
    """One roi -> (segment pair-ids int16 sorted-by-construction,
    W [nseg, ep, 49] float32)."""
    rows = np.unique(Yt)                        # distinct image rows, sorted
    xs = np.unique(Xt)                          # distinct tap columns, sorted
    # greedy cover of xs by [start, start+ep) windows, start even, <= W-ep
    starts = []
    cov = -1
    for x in xs:
        if x > cov:
            s = min(int(x) & ~1, W - ep)
            starts.append(s)
            cov = s + ep - 1
    starts = np.asarray(starts, dtype=np.int32)
    ncov = len(starts)
    # map each tap column -> cover window (first window covering it)
    cov_id = np.searchsorted(starts, Xt, side="right") - 1
    # (windows are disjoint except possibly near the right edge; the
    # found window always covers the tap since cov >= tap for greedy)
    slice_in = Xt - starts[cov_id]
    assert (slice_in >= 0).all() and (slice_in < ep).all()

    row_pos = np.searchsorted(rows, Yt)
    nseg = len(rows) * ncov
    # segment i = row_pos * ncov + cov_id  (row-major; ids ascending)
    seg_ids = (rows[:, None] * (W // 2) + (starts[None, :] >> 1)).ravel()
    tap_seg = (row_pos[:, None] * ncov + cov_id[None, :]).ravel()
    tap_slice = np.broadcast_to(slice_in[None, :], (NTAP, NTAP)).ravel()
    w = (Wy[:, None].astype(np.float64) * Wx[None, :].astype(np.float64)).ravel()

    Wt = np.zeros((nseg, ep, NBIN), dtype=np.float64)
    np.add.at(Wt, (tap_seg, tap_slice, _BIN_OF_TAP), w)
    return seg_ids.astype(np.int16), Wt.astype(np.float32)


def _build_nc(J_prof, ep, use_bf16):
    """SPMD kernel for a per-slot chunk-count profile (tuple of ints)."""
    import concourse.bacc as bacc
    import concourse.tile as tile
    import concourse.mybir as mybir

    K = len(J_prof)
    total_mm = int(ep * sum(J_prof))
    total_cols = int(8 * sum(J_prof))
    ELEM = ep * C
    dt = mybir.dt.bfloat16 if use_bf16 else mybir.dt.float32r
    f32 = mybir.dt.float32
    NROW = NPAIR - ep // 2 + 1

    nc = bacc.Bacc("TRN2", debug=False)
    img_d = nc.declare_dram_parameter("img", [NPAIR, PAIR_ELEMS], dt, isOutput=False)
    idx_d = nc.declare_dram_parameter("idx", [128, total_cols], mybir.dt.int16, isOutput=False)
    wts_d = nc.declare_dram_parameter("wts", [128, total_mm, NBIN], dt, isOutput=False)
    cnt_d = nc.declare_dram_parameter("cnt", [1, K], mybir.dt.int32, isOutput=False)
    out_d = nc.declare_dram_parameter("out", [K, NBIN, C], f32, isOutput=True)

    def img_view():
        ap = img_d[:]
        ap.ap.clear()
        ap.ap.extend([[PAIR_ELEMS, NROW], [1, ELEM]])
        return ap

    with tile.TileContext(nc) as tc:
        with tc.tile_pool(name="meta", bufs=1) as meta_pool, \
             tc.tile_pool(name="g", bufs=3) as gpool, \
             tc.tile_pool(name="w", bufs=3) as wpool, \
             tc.tile_pool(name="o", bufs=4) as opool, \
             tc.tile_pool(name="ps", bufs=6, space="PSUM") as pspool:
            idx_t = meta_pool.tile([128, total_cols], mybir.dt.int16)
            cnt_t = meta_pool.tile([1, K], mybir.dt.int32)
            nc.sync.dma_start(out=idx_t[:], in_=idx_d[:])
            nc.sync.dma_start(out=cnt_t[:], in_=cnt_d[:])

            icol = imm = 0
            for s0 in range(0, K, WBATCH):
                batch = list(range(s0, min(s0 + WBATCH, K)))
                bmm = sum(ep * J_prof[s] for s in batch)
                w_t = wpool.tile([128, bmm, NBIN], dt, tag="w")
                nc.sync.dma_start(out=w_t[:], in_=wts_d[:, imm:imm + bmm, :])
                o_t = opool.tile([NBIN, len(batch), C], f32, tag="o")
                wofs = 0
                for bi, s in enumerate(batch):
                    J = J_prof[s]
                    g_t = gpool.tile([128, J, ELEM], dt, tag="g")
                    reg = nc.gpsimd.alloc_register()
                    nc.gpsimd.reg_load(reg, cnt_t[:, s:s + 1])
                    nc.gpsimd.dma_gather(
                        g_t[:], img_view(), idx_t[:, icol:icol + 8 * J],
                        J * 128, reg, ELEM, elem_step=PAIR_ELEMS,
                    )
                    ps = pspool.tile([NBIN, C], f32, space="PSUM", tag="ps")
                    nmm = J * ep
                    m = 0
                    for j in range(J):
                        for px in range(ep):
                            nc.tensor.matmul(
                                ps[:], lhsT=w_t[:, wofs + j * ep + px, :],
                                rhs=g_t[:, j, px * C:(px + 1) * C],
                                start=(m == 0), stop=(m == nmm - 1),
                            )
                            m += 1
                    nc.scalar.copy(o_t[:, bi, :], ps[:])
                    icol += 8 * J
                    wofs += ep * J
                imm += bmm
                nc.scalar.dma_start(
                    out=out_d[s0:s0 + len(batch)].rearrange("s b c -> b s c"),
                    in_=o_t[:, 0:len(batch), :],
                )
    nc.compile()
    return nc


def kernel(input, rois):
    global LAST_RESULTS
    from concourse.bass_utils import run_bass_kernel_spmd
    import ml_dtypes

    input = np.ascontiguousarray(input, dtype=np.float32)
    rois = np.asarray(rois, dtype=np.float32)

    img_nhwc = np.ascontiguousarray(input.transpose(0, 2, 3, 1))
    np_dt = ml_dtypes.bfloat16 if USE_BF16 else np.float32
    img_pairs = img_nhwc.reshape(N, NPAIR, PAIR_ELEMS).astype(np_dt)

    bidx, Yt, Xt, Wy, Wx = _roi_meta(rois)
    tables = [_roi_tables(Yt[r], Xt[r], Wy[r], Wx[r], EP) for r in range(B)]
    cnts = np.array([len(t[0]) for t in tables])

    # ---- route rois: image n -> cores 2n, 2n+1, sorted by gather size ----
    core_rois = [[] for _ in range(8)]
    for n in range(N):
        rs = np.where(bidx == n)[0]
        rs = rs[np.argsort(-cnts[rs], kind="stable")]
        core_rois[2 * n] = list(rs[0::2])
        core_rois[2 * n + 1] = list(rs[1::2])

    K = max(len(cr) for cr in core_rois)
    J_prof = []
    for s in range(K):
        m = max((cnts[cr[s]] if s < len(cr) else 1) for cr in core_rois)
        J_prof.append(int(-(-int(m) // 128)))
    J_prof = tuple(J_prof)
    total_mm = EP * sum(J_prof)
    total_cols = 8 * sum(J_prof)

    # ---- per-core input arrays ----
    in_maps = []
    for c in range(8):
        cr = core_rois[c]
        idx_arr = np.full((16, total_cols), -1, dtype=np.int16)
        cnt_arr = np.zeros((1, K), dtype=np.int32)
        wts_arr = np.zeros((128, total_mm, NBIN), dtype=np_dt)
        icol = imm = 0
        for s in range(K):
            J = J_prof[s]
            if s < len(cr):
                ids, Wt = tables[cr[s]]
                m = len(ids)
            else:
                ids = np.zeros(1, dtype=np.int16)
                Wt = np.zeros((1, EP, NBIN), dtype=np.float32)
                m = 1
            cnt_arr[0, s] = m
            pad = np.full(J * 128, -1, dtype=np.int16)
            pad[:m] = ids
            idx_arr[:, icol:icol + 8 * J] = pad.reshape(-1, 16).T
            wt = np.zeros((J * 128, EP, NBIN), dtype=np.float32)
            wt[:m] = Wt
            # -> [128, J*EP, 49] slot block: block (j, px) = wt[j*128:(j+1)*128, px]
            blk = wt.reshape(J, 128, EP, NBIN).transpose(1, 0, 2, 3).reshape(128, J * EP, NBIN)
            wts_arr[:, imm:imm + EP * J, :] = blk.astype(np_dt)
            icol += 8 * J
            imm += EP * J
        in_maps.append({
            "img": img_pairs[c // 2],
            "idx": np.tile(idx_arr, (8, 1)),
            "wts": wts_arr,
            "cnt": cnt_arr,
        })

    key = (J_prof, EP, USE_BF16)
    if key not in _NC_CACHE:
        _NC_CACHE[key] = _build_nc(J_prof, EP, USE_BF16)
    nc = _NC_CACHE[key]

    res = run_bass_kernel_spmd(nc, in_maps, core_ids=list(range(8)))
    LAST_RESULTS = res

    out = np.zeros((B, C, PH, PW), dtype=np.float32)
    for c in range(8):
        ores = res.results[c]["out"]  # [K, 49, 256]
        for s, rid in enumerate(core_rois[c]):
            out[rid] = ores[s].reshape(PH, PW, C).transpose(2, 0, 1)
    return out
